# revision 31
# baseline (speedup 1.0000x reference)
"""Bahdanau attention Trainium2 Bass kernel.

Problem (hardcoded): B=32, S=2048, ENC=DEC=1024, fp32.
  enc_t = encoder_outputs @ W_enc + b_enc          # (B,S,D)
  dec_t = decoder_hidden @ W_dec + b_dec           # (B,D)
  combined = tanh(enc_t + dec_t[:,None,:])         # (B,S,D)
  scores = combined . v  (+ b_v)                   # (B,S)  (b_v drops out of softmax)
  attn = softmax(scores, axis=1)                   # (B,S)
  context = attn . encoder_outputs                 # (B,E)
returns (context, attn)

Sharding: data-parallel over batch across 8 NeuronCores (4 batches/core),
projection matrices replicated. No collectives.

Per-core dataflow (all matmuls in float32r = single-pass fp32 at 1 cyc/row):
  - enc natural tiles [s128, e] are PE-transposed to encT [e128, s] tiles,
    pipelined one r-block ahead so the PSUM->SBUF copies hide under matmuls
  - enc_tT[d, r] = sum_e W_enc[e,d]^T encT[e,r] accumulated in PSUM
  - ScalarE applies tanh(psum + (dec_t[d,b]+b_enc+b_dec)) in one pass (per-
    partition bias) -> combined [d128, r]
  - scores[1, r] = v . combined via PE over partitions
  - per-batch softmax along the free dim; attn transposed back via PE
  - context[1, e] = sum_s attnT[s,1] . enc[s, e] (second streamed enc read),
    emitted one batch late so the softmax latency chain hides under the next
    batch's matmuls
"""

import numpy as np
from contextlib import ExitStack

import concourse.bass as bass
import concourse.tile as tile
from concourse import bacc, mybir
from concourse.bass import ts
from concourse.bass_utils import run_bass_kernel_spmd
from concourse.masks import make_identity

N_CORES = 8
B = 32
B_LOC = B // N_CORES   # 4
S = 2048
E = 1024               # ENC
D = 1024               # DEC
P = 128
RB = 512               # r-block (moving free dim of main matmuls)
NRB = S // RB          # 4 r-blocks per batch
NSUB = RB // P         # 4 s-subtiles per r-block
NE = E // P            # 8 e-chunks
ND = D // P            # 8 d-tiles
NSC = S // P           # 16 s-chunks (context phase)

F32 = mybir.dt.float32
F32R = mybir.dt.float32r

_cache = {}


def _build():
    nc = bacc.Bacc("TRN2", target_bir_lowering=False, debug=False)

    dec = nc.dram_tensor("dec", [B_LOC, D], F32R, kind="ExternalInput").ap()
    enc = nc.dram_tensor("enc", [B_LOC, S, E], F32R, kind="ExternalInput").ap()
    w_enc = nc.dram_tensor("w_enc", [E, D], F32R, kind="ExternalInput").ap()
    b_enc = nc.dram_tensor("b_enc", [D], F32, kind="ExternalInput").ap()
    w_dec = nc.dram_tensor("w_dec", [D, D], F32R, kind="ExternalInput").ap()
    b_dec = nc.dram_tensor("b_dec", [D], F32, kind="ExternalInput").ap()
    v_in = nc.dram_tensor("v", [D], F32R, kind="ExternalInput").ap()
    ctx_o = nc.dram_tensor("ctx", [B_LOC, E], F32, kind="ExternalOutput").ap()
    attn_o = nc.dram_tensor("attn", [B_LOC, S], F32, kind="ExternalOutput").ap()

    with ExitStack() as ec:
        tc = ec.enter_context(tile.TileContext(nc))
        persist = ec.enter_context(tc.tile_pool(name="persist", bufs=1))
        nat_pool = ec.enter_context(tc.tile_pool(name="nat", bufs=2))
        tT_pool = ec.enter_context(tc.tile_pool(name="tT", bufs=2))
        comb_pool = ec.enter_context(tc.tile_pool(name="comb", bufs=2))
        nat2_pool = ec.enter_context(tc.tile_pool(name="nat2", bufs=2))
        wdec_pool = ec.enter_context(tc.tile_pool(name="wdec", bufs=2))
        small = ec.enter_context(tc.tile_pool(name="small", bufs=2))
        ps_tp = ec.enter_context(tc.tile_pool(name="ps_tp", bufs=2, space="PSUM"))
        ps_main = ec.enter_context(tc.tile_pool(name="ps_main", bufs=3, space="PSUM"))
        ps_sc = ec.enter_context(tc.tile_pool(name="ps_sc", bufs=1, space="PSUM"))
        ps_ctx = ec.enter_context(tc.tile_pool(name="ps_ctx", bufs=1, space="PSUM"))

        # ---- identity (for PE transposes) -------------------------------
        id_f = persist.tile([P, P], F32, tag="idf")
        make_identity(nc, id_f[:])
        id_sb = persist.tile([P, P], F32R, tag="id")
        nc.vector.tensor_copy(id_sb[:], id_f[:])
        id_r = id_sb[:]

        # persistent tiles (DMAs emitted below, after the first enc block's)
        w_sb = persist.tile([P, NE, D], F32R, tag="w")
        v_sb = persist.tile([P, ND], F32R, tag="v")
        bias_eb = persist.tile([P, ND], F32, tag="be")
        be_tmp = persist.tile([P, ND], F32, tag="bd")
        decT = persist.tile([P, B_LOC, NE], F32R, tag="decT")
        bias_tot = persist.tile([P, ND, B_LOC], F32, tag="btot")

        # ---- per-block emission helpers ---------------------------------
        def emit_transposes(b, rb):
            """DMA natural enc rows and PE-transpose them to encT layout."""
            nat_t = nat_pool.tile([P, NSUB, E], F32R, tag="nat", name=f"nat_{b}_{rb}")
            src = enc[b, ts(rb, RB), :].rearrange("(a p) e -> p a e", p=P)
            for sub in range(NSUB):
                nc.sync.dma_start(nat_t[:, sub, :], src[:, sub, :])
            tT_t = tT_pool.tile([P, NE, RB], F32R, tag="tT", name=f"tT_{b}_{rb}")
            for ech in range(NE):
                ps_t = ps_tp.tile(
                    [P, NSUB, P], F32R, tag="tp", name=f"ps_t_{b}_{rb}_{ech}"
                )
                for sub in range(NSUB):
                    nc.tensor.transpose(
                        ps_t[:, sub, :], nat_t[:, sub, ts(ech, P)], id_r
                    )
                nc.vector.tensor_copy(
                    tT_t[:, ech, :], ps_t[:].rearrange("p a q -> p (a q)")
                )
            return tT_t

        def emit_mains(b, rb, tT_t, scores_b):
            """Projection matmuls + tanh + v-scores for one r-block."""
            comb_t = comb_pool.tile(
                [P, ND, RB], F32R, tag="comb", name=f"comb_{b}_{rb}"
            )
            for dg in range(ND // 2):
                ps_m = [
                    ps_main.tile(
                        [P, RB], F32, tag="main", name=f"psm_{b}_{rb}_{dg}_{j}"
                    )
                    for j in range(2)
                ]
                for ech in range(NE):
                    for j in range(2):
                        dt = dg * 2 + j
                        nc.tensor.matmul(
                            ps_m[j][:],
                            w_sb[:, ech, ts(dt, P)],
                            tT_t[:, ech, :],
                            start=(ech == 0),
                            stop=(ech == NE - 1),
                        )
                for j in range(2):
                    dt = dg * 2 + j
                    nc.scalar.activation(
                        comb_t[:, dt, :],
                        ps_m[j][:],
                        mybir.ActivationFunctionType.Tanh,
                        bias=bias_tot[:, dt, b : b + 1],
                    )
            ps_s = ps_sc.tile([1, RB], F32, tag="sc", name=f"ps_s_{b}_{rb}")
            for dt in range(ND):
                nc.tensor.matmul(
                    ps_s[:],
                    v_sb[:, dt : dt + 1],
                    comb_t[:, dt, :],
                    start=(dt == 0),
                    stop=(dt == ND - 1),
                )
            nc.vector.tensor_copy(scores_b[0:1, ts(rb, RB)], ps_s[:])

        def emit_softmax(b, scores_b):
            negmax = small.tile([1, 1], F32, tag="negmax", name=f"negmax_{b}")
            nc.vector.reduce_max(
                negmax[:], scores_b[:], axis=mybir.AxisListType.X, negate=True
            )
            ssum = small.tile([1, 1], F32, tag="ssum", name=f"ssum_{b}")
            nc.scalar.activation(
                scores_b[:],
                scores_b[:],
                mybir.ActivationFunctionType.Exp,
                bias=negmax[0:1, 0:1],
                accum_out=ssum[0:1, 0:1],
            )
            rsum = small.tile([1, 1], F32, tag="rsum", name=f"rsum_{b}")
            nc.vector.reciprocal(rsum[:], ssum[:])
            attn_n = scores_b
            nc.vector.tensor_scalar_mul(attn_n[:], scores_b[:], rsum[0:1, 0:1])
            nc.sync.dma_start(attn_o[b : b + 1, :], attn_n[:])
            return attn_n

        def emit_ctx(b, attn_n):
            """attn transpose + context matmuls for batch b (one batch late)."""
            ps_at = ps_tp.tile([P, NSC], F32, tag="tp", name=f"ps_at_{b}")
            for c in range(NSC):
                nc.tensor.transpose(
                    ps_at[:, c : c + 1],
                    attn_n[0:1, ts(c, P)],
                    id_f[0:1, 0:1],
                )
            attnT = small.tile([P, NSC], F32R, tag="attnT", name=f"attnT_{b}")
            nc.vector.tensor_copy(attnT[:], ps_at[:])

            ps_c = ps_ctx.tile([1, E], F32, tag="ctxp", name=f"ps_c_{b}")
            for rb in range(NRB):
                nat2_t = nat2_pool.tile(
                    [P, NSUB, E], F32R, tag="nat2", name=f"nat2_{b}_{rb}"
                )
                src2 = enc[b, ts(rb, RB), :].rearrange("(a p) e -> p a e", p=P)
                for sub in range(NSUB):
                    nc.sync.dma_start(nat2_t[:, sub, :], src2[:, sub, :])
                for sub in range(NSUB):
                    c = rb * NSUB + sub
                    for half in range(2):
                        nc.tensor.matmul(
                            ps_c[0:1, ts(half, 512)],
                            attnT[:, c : c + 1],
                            nat2_t[:, sub, ts(half, 512)],
                            start=(c == 0),
                            stop=(c == NSC - 1),
                        )
            ctx_sb = small.tile([1, E], F32, tag="ctx_sb", name=f"ctx_sb_{b}")
            nc.vector.tensor_copy(ctx_sb[:], ps_c[:])
            nc.sync.dma_start(ctx_o[b : b + 1, :], ctx_sb[:])

        # ---- prologue: first enc block first, then weights/constants ----
        blocks = [(b, rb) for b in range(B_LOC) for rb in range(NRB)]
        tTs = {blocks[0]: emit_transposes(*blocks[0])}

        for ech in range(NE):
            nc.sync.dma_start(w_sb[:, ech, :], w_enc[ts(ech, P), :])
        nc.sync.dma_start(v_sb[:], v_in.rearrange("(c p) -> p c", p=P))
        nc.sync.dma_start(bias_eb[:], b_enc.rearrange("(c p) -> p c", p=P))
        nc.sync.dma_start(be_tmp[:], b_dec.rearrange("(c p) -> p c", p=P))
        nc.vector.tensor_add(bias_eb[:], bias_eb[:], be_tmp[:])
        for bb in range(B_LOC):
            nc.sync.dma_start(
                decT[:, bb, :], dec[bb, :].rearrange("(c p) -> p c", p=P)
            )

        def emit_dec_projection():
            # One psum tile per d-tile: PSUM has_written state is per-BANK,
            # so interleaved accumulation groups in one bank lose terms.
            for dt in range(ND):
                wd_t = wdec_pool.tile(
                    [P, NE, P], F32R, tag="wdec", name=f"wdt_{dt}"
                )
                nc.sync.dma_start(
                    wd_t[:], w_dec[:, ts(dt, P)].rearrange("(c p) d -> p c d", p=P)
                )
                ps_d = ps_tp.tile([P, B_LOC], F32, tag="tp", name=f"psd_{dt}")
                for ech in range(NE):
                    nc.tensor.matmul(
                        ps_d[:],
                        wd_t[:, ech, :],
                        decT[:, :, ech],
                        start=(ech == 0),
                        stop=(ech == NE - 1),
                    )
                # bias_total[d, dt, b] = dec_t[d, b] + (b_enc + b_dec)[d]
                nc.vector.tensor_scalar_add(
                    bias_tot[:, dt, :], ps_d[:], bias_eb[:, dt : dt + 1]
                )

        # ---- main loop: transposes pipelined one r-block ahead ----------
        scores = {}
        pending = None
        last = blocks[-1]
        for i, (b, rb) in enumerate(blocks):
            if rb == 0:
                scores[b] = small.tile([1, S], F32, tag="scores", name=f"scores_{b}")
            if i + 1 < len(blocks):
                tTs[blocks[i + 1]] = emit_transposes(*blocks[i + 1])
            if i == 0:
                # dec projection here: its W_dec stream queues behind the
                # first two enc blocks, and its matmuls warm the PE early.
                emit_dec_projection()
            emit_mains(b, rb, tTs.pop((b, rb)), scores[b])
            # context of the PREVIOUS batch, early in this batch's phase B:
            # its softmax finished during the last blocks of the previous
            # batch, so PE never stalls on the chain, and the tail shrinks.
            if rb == 0 and pending is not None and b < B_LOC - 1:
                emit_ctx(*pending)
                pending = None
            if (b, rb) == last:
                # keep PE busy under the last softmax chain with the
                # second-to-last batch's context phase
                if pending is not None:
                    emit_ctx(*pending)
                pending = (b, emit_softmax(b, scores.pop(b)))
            elif rb == NRB - 1:
                pending = (b, emit_softmax(b, scores.pop(b)))
        emit_ctx(*pending)

    nc.compile()
    return nc


def kernel(
    decoder_hidden,
    encoder_outputs,
    W_enc,
    b_enc,
    W_dec,
    b_dec,
    v,
    b_v=None,
    trace=False,
    **_unused,
):
    decoder_hidden = np.ascontiguousarray(np.asarray(decoder_hidden, dtype=np.float32))
    encoder_outputs = np.ascontiguousarray(np.asarray(encoder_outputs, dtype=np.float32))
    W_enc = np.ascontiguousarray(np.asarray(W_enc, dtype=np.float32))
    b_enc = np.ascontiguousarray(np.asarray(b_enc, dtype=np.float32))
    W_dec = np.ascontiguousarray(np.asarray(W_dec, dtype=np.float32))
    b_dec = np.ascontiguousarray(np.asarray(b_dec, dtype=np.float32))
    v = np.ascontiguousarray(np.asarray(v, dtype=np.float32))

    if "nc" not in _cache:
        _cache["nc"] = _build()
    nc = _cache["nc"]

    in_maps = []
    for c in range(N_CORES):
        sl = slice(c * B_LOC, (c + 1) * B_LOC)
        in_maps.append(
            {
                "dec": decoder_hidden[sl],
                "enc": encoder_outputs[sl],
                "w_enc": W_enc,
                "b_enc": b_enc,
                "w_dec": W_dec,
                "b_dec": b_dec,
                "v": v,
            }
        )

    res = run_bass_kernel_spmd(
        nc, in_maps, core_ids=list(range(N_CORES)), trace=trace
    )
    _cache["exec_time_ns"] = res.exec_time_ns
    _cache["results"] = res.results
    if res.instructions_and_trace:
        _cache["trace_path"] = res.instructions_and_trace[1]
    context = np.concatenate([r["ctx"] for r in res.results], axis=0)
    attn = np.concatenate([r["attn"] for r in res.results], axis=0)
    return (context, attn)


if __name__ == "__main__":
    rng = np.random.default_rng(0)
    inputs = {
        "decoder_hidden": rng.standard_normal((B, D)).astype(np.float32),
        "encoder_outputs": rng.standard_normal((B, S, E)).astype(np.float32),
        "W_enc": rng.uniform(-1 / 32, 1 / 32, (E, D)).astype(np.float32),
        "b_enc": rng.uniform(-1 / 32, 1 / 32, (D,)).astype(np.float32),
        "W_dec": rng.uniform(-1 / 32, 1 / 32, (D, D)).astype(np.float32),
        "b_dec": rng.uniform(-1 / 32, 1 / 32, (D,)).astype(np.float32),
        "v": rng.uniform(-1 / 32, 1 / 32, (D,)).astype(np.float32),
        "b_v": np.float32(0.01),
    }
    ctx, attn = kernel(**inputs)
    print("ctx", ctx.shape, "attn", attn.shape)


# revision 35
# speedup vs baseline: 1.0082x; 1.0082x over previous
"""Bahdanau attention Trainium2 Bass kernel.

Problem (hardcoded): B=32, S=2048, ENC=DEC=1024, fp32.
  enc_t = encoder_outputs @ W_enc + b_enc          # (B,S,D)
  dec_t = decoder_hidden @ W_dec + b_dec           # (B,D)
  combined = tanh(enc_t + dec_t[:,None,:])         # (B,S,D)
  scores = combined . v  (+ b_v)                   # (B,S)  (b_v drops out of softmax)
  attn = softmax(scores, axis=1)                   # (B,S)
  context = attn . encoder_outputs                 # (B,E)
returns (context, attn)

Sharding: data-parallel over batch across 8 NeuronCores (4 batches/core),
projection matrices replicated. No collectives.

Per-core dataflow (all matmuls in float32r = single-pass fp32 at 1 cyc/row):
  - enc natural tiles [s128, e] are PE-transposed to encT [e128, s] tiles,
    pipelined one r-block ahead so the PSUM->SBUF copies hide under matmuls
  - enc_tT[d, r] = sum_e W_enc[e,d]^T encT[e,r] accumulated in PSUM
  - ScalarE applies tanh(psum + (dec_t[d,b]+b_enc+b_dec)) in one pass (per-
    partition bias) -> combined [d128, r]
  - scores[1, r] = v . combined via PE over partitions
  - per-batch softmax along the free dim; attn transposed back via PE
  - context[1, e] = sum_s attnT[s,1] . enc[s, e] (second streamed enc read),
    emitted one batch late so the softmax latency chain hides under the next
    batch's matmuls
"""

import numpy as np
from contextlib import ExitStack

import concourse.bass as bass
import concourse.tile as tile
from concourse import bacc, mybir
from concourse.bass import ts
from concourse.bass_utils import run_bass_kernel_spmd
from concourse.masks import make_identity

N_CORES = 8
B = 32
B_LOC = B // N_CORES   # 4
S = 2048
E = 1024               # ENC
D = 1024               # DEC
P = 128
RB = 512               # r-block (moving free dim of main matmuls)
NRB = S // RB          # 4 r-blocks per batch
NSUB = RB // P         # 4 s-subtiles per r-block
NE = E // P            # 8 e-chunks
ND = D // P            # 8 d-tiles
NSC = S // P           # 16 s-chunks (context phase)

F32 = mybir.dt.float32
F32R = mybir.dt.float32r

_cache = {}


def _build():
    nc = bacc.Bacc("TRN2", target_bir_lowering=False, debug=False)

    dec = nc.dram_tensor("dec", [B_LOC, D], F32R, kind="ExternalInput").ap()
    enc = nc.dram_tensor("enc", [B_LOC, S, E], F32R, kind="ExternalInput").ap()
    w_enc = nc.dram_tensor("w_enc", [E, D], F32R, kind="ExternalInput").ap()
    b_enc = nc.dram_tensor("b_enc", [D], F32, kind="ExternalInput").ap()
    w_dec = nc.dram_tensor("w_dec", [D, D], F32R, kind="ExternalInput").ap()
    b_dec = nc.dram_tensor("b_dec", [D], F32, kind="ExternalInput").ap()
    v_in = nc.dram_tensor("v", [D], F32R, kind="ExternalInput").ap()
    ctx_o = nc.dram_tensor("ctx", [B_LOC, E], F32, kind="ExternalOutput").ap()
    attn_o = nc.dram_tensor("attn", [B_LOC, S], F32, kind="ExternalOutput").ap()

    with ExitStack() as ec:
        tc = ec.enter_context(tile.TileContext(nc))
        persist = ec.enter_context(tc.tile_pool(name="persist", bufs=1))
        nat_pool = ec.enter_context(tc.tile_pool(name="nat", bufs=2))
        tT_pool = ec.enter_context(tc.tile_pool(name="tT", bufs=2))
        comb_pool = ec.enter_context(tc.tile_pool(name="comb", bufs=2))
        nat2_pool = ec.enter_context(tc.tile_pool(name="nat2", bufs=2))
        wdec_pool = ec.enter_context(tc.tile_pool(name="wdec", bufs=2))
        small = ec.enter_context(tc.tile_pool(name="small", bufs=2))
        ps_tp = ec.enter_context(tc.tile_pool(name="ps_tp", bufs=2, space="PSUM"))
        ps_main = ec.enter_context(tc.tile_pool(name="ps_main", bufs=3, space="PSUM"))
        ps_sc = ec.enter_context(tc.tile_pool(name="ps_sc", bufs=1, space="PSUM"))
        ps_ctx = ec.enter_context(tc.tile_pool(name="ps_ctx", bufs=1, space="PSUM"))

        # ---- identity (for PE transposes) -------------------------------
        id_f = persist.tile([P, P], F32, tag="idf")
        make_identity(nc, id_f[:])
        id_sb = persist.tile([P, P], F32R, tag="id")
        nc.vector.tensor_copy(id_sb[:], id_f[:])
        id_r = id_sb[:]

        # persistent tiles (DMAs emitted below, after the first enc block's)
        w_sb = persist.tile([P, NE, D], F32R, tag="w")
        v_sb = persist.tile([P, ND], F32R, tag="v")
        bias_eb = persist.tile([P, ND], F32, tag="be")
        be_tmp = persist.tile([P, ND], F32, tag="bd")
        decT = persist.tile([P, B_LOC, NE], F32R, tag="decT")
        bias_tot = persist.tile([P, ND, B_LOC], F32, tag="btot")

        # ---- per-block emission helpers ---------------------------------
        def emit_transposes(b, rb):
            """DMA natural enc rows and PE-transpose them to encT layout."""
            nat_t = nat_pool.tile([P, NSUB, E], F32R, tag="nat", name=f"nat_{b}_{rb}")
            src = enc[b, ts(rb, RB), :].rearrange("(a p) e -> p a e", p=P)
            for sub in range(NSUB):
                nc.sync.dma_start(nat_t[:, sub, :], src[:, sub, :])
            tT_t = tT_pool.tile([P, NE, RB], F32R, tag="tT", name=f"tT_{b}_{rb}")
            for ech in range(NE):
                ps_t = ps_tp.tile(
                    [P, NSUB, P], F32R, tag="tp", name=f"ps_t_{b}_{rb}_{ech}"
                )
                for sub in range(NSUB):
                    nc.tensor.transpose(
                        ps_t[:, sub, :], nat_t[:, sub, ts(ech, P)], id_r
                    )
                nc.vector.tensor_copy(
                    tT_t[:, ech, :], ps_t[:].rearrange("p a q -> p (a q)")
                )
            return tT_t

        def emit_mains(b, rb, tT_t, scores_b):
            """Projection matmuls + tanh + v-scores for one r-block."""
            comb_t = comb_pool.tile(
                [P, ND, RB], F32R, tag="comb", name=f"comb_{b}_{rb}"
            )
            for dg in range(ND // 2):
                ps_m = [
                    ps_main.tile(
                        [P, RB], F32, tag="main", name=f"psm_{b}_{rb}_{dg}_{j}"
                    )
                    for j in range(2)
                ]
                for ech in range(NE):
                    for j in range(2):
                        dt = dg * 2 + j
                        nc.tensor.matmul(
                            ps_m[j][:],
                            w_sb[:, ech, ts(dt, P)],
                            tT_t[:, ech, :],
                            start=(ech == 0),
                            stop=(ech == NE - 1),
                        )
                for j in range(2):
                    dt = dg * 2 + j
                    nc.scalar.activation(
                        comb_t[:, dt, :],
                        ps_m[j][:],
                        mybir.ActivationFunctionType.Tanh,
                        bias=bias_tot[:, dt, b : b + 1],
                    )
            ps_s = ps_sc.tile([1, RB], F32, tag="sc", name=f"ps_s_{b}_{rb}")
            for dt in range(ND):
                nc.tensor.matmul(
                    ps_s[:],
                    v_sb[:, dt : dt + 1],
                    comb_t[:, dt, :],
                    start=(dt == 0),
                    stop=(dt == ND - 1),
                )
            nc.vector.tensor_copy(scores_b[0:1, ts(rb, RB)], ps_s[:])

        def emit_softmax(b, scores_b):
            negmax = small.tile([1, 1], F32, tag="negmax", name=f"negmax_{b}")
            nc.vector.reduce_max(
                negmax[:], scores_b[:], axis=mybir.AxisListType.X, negate=True
            )
            ssum = small.tile([1, 1], F32, tag="ssum", name=f"ssum_{b}")
            nc.scalar.activation(
                scores_b[:],
                scores_b[:],
                mybir.ActivationFunctionType.Exp,
                bias=negmax[0:1, 0:1],
                accum_out=ssum[0:1, 0:1],
            )
            rsum = small.tile([1, 1], F32, tag="rsum", name=f"rsum_{b}")
            nc.vector.reciprocal(rsum[:], ssum[:])
            attn_n = scores_b
            nc.vector.tensor_scalar_mul(attn_n[:], scores_b[:], rsum[0:1, 0:1])
            nc.sync.dma_start(attn_o[b : b + 1, :], attn_n[:])
            return attn_n

        def emit_ctx(b, attn_n):
            """attn transpose + context matmuls for batch b (one batch late)."""
            ps_at = ps_tp.tile([P, NSC], F32, tag="tp", name=f"ps_at_{b}")
            for c in range(NSC):
                nc.tensor.transpose(
                    ps_at[:, c : c + 1],
                    attn_n[0:1, ts(c, P)],
                    id_f[0:1, 0:1],
                )
            attnT = small.tile([P, NSC], F32R, tag="attnT", name=f"attnT_{b}")
            nc.vector.tensor_copy(attnT[:], ps_at[:])

            ps_c = ps_ctx.tile([1, E], F32, tag="ctxp", name=f"ps_c_{b}")
            for rb in range(NRB):
                nat2_t = nat2_pool.tile(
                    [P, NSUB, E], F32R, tag="nat2", name=f"nat2_{b}_{rb}"
                )
                src2 = enc[b, ts(rb, RB), :].rearrange("(a p) e -> p a e", p=P)
                for sub in range(NSUB):
                    nc.sync.dma_start(nat2_t[:, sub, :], src2[:, sub, :])
                for sub in range(NSUB):
                    c = rb * NSUB + sub
                    for half in range(2):
                        nc.tensor.matmul(
                            ps_c[0:1, ts(half, 512)],
                            attnT[:, c : c + 1],
                            nat2_t[:, sub, ts(half, 512)],
                            start=(c == 0),
                            stop=(c == NSC - 1),
                        )
            ctx_sb = small.tile([1, E], F32, tag="ctx_sb", name=f"ctx_sb_{b}")
            nc.vector.tensor_copy(ctx_sb[:], ps_c[:])
            nc.sync.dma_start(ctx_o[b : b + 1, :], ctx_sb[:])

        # ---- prologue: first two enc blocks first, then weights ---------
        blocks = [(b, rb) for b in range(B_LOC) for rb in range(NRB)]
        tTs = {
            blocks[0]: emit_transposes(*blocks[0]),
            blocks[1]: emit_transposes(*blocks[1]),
        }

        for ech in range(NE):
            nc.sync.dma_start(w_sb[:, ech, :], w_enc[ts(ech, P), :])
        nc.sync.dma_start(v_sb[:], v_in.rearrange("(c p) -> p c", p=P))
        nc.sync.dma_start(bias_eb[:], b_enc.rearrange("(c p) -> p c", p=P))
        nc.sync.dma_start(be_tmp[:], b_dec.rearrange("(c p) -> p c", p=P))
        nc.vector.tensor_add(bias_eb[:], bias_eb[:], be_tmp[:])
        for bb in range(B_LOC):
            nc.sync.dma_start(
                decT[:, bb, :], dec[bb, :].rearrange("(c p) -> p c", p=P)
            )

        def emit_dec_projection():  # noqa: E306
            # One psum tile per d-tile: PSUM has_written state is per-BANK,
            # so interleaved accumulation groups in one bank lose terms.
            for dt in range(ND):
                wd_t = wdec_pool.tile(
                    [P, NE, P], F32R, tag="wdec", name=f"wdt_{dt}"
                )
                nc.sync.dma_start(
                    wd_t[:], w_dec[:, ts(dt, P)].rearrange("(c p) d -> p c d", p=P)
                )
                ps_d = ps_tp.tile([P, B_LOC], F32, tag="tp", name=f"psd_{dt}")
                for ech in range(NE):
                    nc.tensor.matmul(
                        ps_d[:],
                        wd_t[:, ech, :],
                        decT[:, :, ech],
                        start=(ech == 0),
                        stop=(ech == NE - 1),
                    )
                # bias_total[d, dt, b] = dec_t[d, b] + (b_enc + b_dec)[d]
                nc.vector.tensor_scalar_add(
                    bias_tot[:, dt, :], ps_d[:], bias_eb[:, dt : dt + 1]
                )

        emit_dec_projection()

        # ---- main loop: transposes pipelined one r-block ahead ----------
        scores = {}
        pending = None
        last = blocks[-1]
        for i, (b, rb) in enumerate(blocks):
            if rb == 0:
                scores[b] = small.tile([1, S], F32, tag="scores", name=f"scores_{b}")
            if i + 2 < len(blocks):
                tTs[blocks[i + 2]] = emit_transposes(*blocks[i + 2])
            emit_mains(b, rb, tTs.pop((b, rb)), scores[b])
            # context of the PREVIOUS batch, early in this batch's phase B:
            # its softmax finished during the last blocks of the previous
            # batch, so PE never stalls on the chain, and the tail shrinks.
            if rb == 0 and pending is not None and b < B_LOC - 1:
                emit_ctx(*pending)
                pending = None
            if (b, rb) == last:
                # keep PE busy under the last softmax chain with the
                # second-to-last batch's context phase
                if pending is not None:
                    emit_ctx(*pending)
                pending = (b, emit_softmax(b, scores.pop(b)))
            elif rb == NRB - 1:
                pending = (b, emit_softmax(b, scores.pop(b)))
        emit_ctx(*pending)

    nc.compile()
    return nc


def kernel(
    decoder_hidden,
    encoder_outputs,
    W_enc,
    b_enc,
    W_dec,
    b_dec,
    v,
    b_v=None,
    trace=False,
    **_unused,
):
    decoder_hidden = np.ascontiguousarray(np.asarray(decoder_hidden, dtype=np.float32))
    encoder_outputs = np.ascontiguousarray(np.asarray(encoder_outputs, dtype=np.float32))
    W_enc = np.ascontiguousarray(np.asarray(W_enc, dtype=np.float32))
    b_enc = np.ascontiguousarray(np.asarray(b_enc, dtype=np.float32))
    W_dec = np.ascontiguousarray(np.asarray(W_dec, dtype=np.float32))
    b_dec = np.ascontiguousarray(np.asarray(b_dec, dtype=np.float32))
    v = np.ascontiguousarray(np.asarray(v, dtype=np.float32))

    if "nc" not in _cache:
        _cache["nc"] = _build()
    nc = _cache["nc"]

    in_maps = []
    for c in range(N_CORES):
        sl = slice(c * B_LOC, (c + 1) * B_LOC)
        in_maps.append(
            {
                "dec": decoder_hidden[sl],
                "enc": encoder_outputs[sl],
                "w_enc": W_enc,
                "b_enc": b_enc,
                "w_dec": W_dec,
                "b_dec": b_dec,
                "v": v,
            }
        )

    res = run_bass_kernel_spmd(
        nc, in_maps, core_ids=list(range(N_CORES)), trace=trace
    )
    _cache["exec_time_ns"] = res.exec_time_ns
    _cache["results"] = res.results
    if res.instructions_and_trace:
        _cache["trace_path"] = res.instructions_and_trace[1]
    context = np.concatenate([r["ctx"] for r in res.results], axis=0)
    attn = np.concatenate([r["attn"] for r in res.results], axis=0)
    return (context, attn)


if __name__ == "__main__":
    rng = np.random.default_rng(0)
    inputs = {
        "decoder_hidden": rng.standard_normal((B, D)).astype(np.float32),
        "encoder_outputs": rng.standard_normal((B, S, E)).astype(np.float32),
        "W_enc": rng.uniform(-1 / 32, 1 / 32, (E, D)).astype(np.float32),
        "b_enc": rng.uniform(-1 / 32, 1 / 32, (D,)).astype(np.float32),
        "W_dec": rng.uniform(-1 / 32, 1 / 32, (D, D)).astype(np.float32),
        "b_dec": rng.uniform(-1 / 32, 1 / 32, (D,)).astype(np.float32),
        "v": rng.uniform(-1 / 32, 1 / 32, (D,)).astype(np.float32),
        "b_v": np.float32(0.01),
    }
    ctx, attn = kernel(**inputs)
    print("ctx", ctx.shape, "attn", attn.shape)


# revision 37
# speedup vs baseline: 1.0351x; 1.0267x over previous
"""Bahdanau attention Trainium2 Bass kernel.

Problem (hardcoded): B=32, S=2048, ENC=DEC=1024, fp32.
  enc_t = encoder_outputs @ W_enc + b_enc          # (B,S,D)
  dec_t = decoder_hidden @ W_dec + b_dec           # (B,D)
  combined = tanh(enc_t + dec_t[:,None,:])         # (B,S,D)
  scores = combined . v  (+ b_v)                   # (B,S)  (b_v drops out of softmax)
  attn = softmax(scores, axis=1)                   # (B,S)
  context = attn . encoder_outputs                 # (B,E)
returns (context, attn)

Sharding: data-parallel over batch across 8 NeuronCores (4 batches/core),
projection matrices replicated. No collectives.

Per-core dataflow (all matmuls in float32r = single-pass fp32 at 1 cyc/row):
  - enc natural tiles [s128, e] are PE-transposed to encT [e128, s] tiles,
    pipelined one r-block ahead so the PSUM->SBUF copies hide under matmuls
  - enc_tT[d, r] = sum_e W_enc[e,d]^T encT[e,r] accumulated in PSUM
  - ScalarE applies tanh(psum + (dec_t[d,b]+b_enc+b_dec)) in one pass (per-
    partition bias) -> combined [d128, r]
  - scores[1, r] = v . combined via PE over partitions
  - per-batch softmax along the free dim; attn transposed back via PE
  - context[1, e] = sum_s attnT[s,1] . enc[s, e] (second streamed enc read),
    emitted one batch late so the softmax latency chain hides under the next
    batch's matmuls
"""

import numpy as np
from contextlib import ExitStack

import concourse.bass as bass
import concourse.tile as tile
from concourse import bacc, mybir
from concourse.bass import ts
from concourse.bass_utils import run_bass_kernel_spmd
from concourse.masks import make_identity

N_CORES = 8
B = 32
B_LOC = B // N_CORES   # 4
S = 2048
E = 1024               # ENC
D = 1024               # DEC
P = 128
RB = 512               # r-block (moving free dim of main matmuls)
NRB = S // RB          # 4 r-blocks per batch
NSUB = RB // P         # 4 s-subtiles per r-block
NE = E // P            # 8 e-chunks
ND = D // P            # 8 d-tiles
NSC = S // P           # 16 s-chunks (context phase)

F32 = mybir.dt.float32
F32R = mybir.dt.float32r

_cache = {}


def _build():
    nc = bacc.Bacc("TRN2", target_bir_lowering=False, debug=False)

    dec = nc.dram_tensor("dec", [B_LOC, D], F32R, kind="ExternalInput").ap()
    enc = nc.dram_tensor("enc", [B_LOC, S, E], F32R, kind="ExternalInput").ap()
    w_enc = nc.dram_tensor("w_enc", [E, D], F32R, kind="ExternalInput").ap()
    b_enc = nc.dram_tensor("b_enc", [D], F32, kind="ExternalInput").ap()
    w_dec = nc.dram_tensor("w_dec", [D, D], F32R, kind="ExternalInput").ap()
    b_dec = nc.dram_tensor("b_dec", [D], F32, kind="ExternalInput").ap()
    v_in = nc.dram_tensor("v", [D], F32R, kind="ExternalInput").ap()
    ctx_o = nc.dram_tensor("ctx", [B_LOC, E], F32, kind="ExternalOutput").ap()
    attn_o = nc.dram_tensor("attn", [B_LOC, S], F32, kind="ExternalOutput").ap()

    with ExitStack() as ec:
        tc = ec.enter_context(tile.TileContext(nc))
        persist = ec.enter_context(tc.tile_pool(name="persist", bufs=1))
        nat_pool = ec.enter_context(tc.tile_pool(name="nat", bufs=2))
        tT_pool = ec.enter_context(tc.tile_pool(name="tT", bufs=2))
        comb_pool = ec.enter_context(tc.tile_pool(name="comb", bufs=2))
        nat2_pool = ec.enter_context(tc.tile_pool(name="nat2", bufs=2))
        wdec_pool = ec.enter_context(tc.tile_pool(name="wdec", bufs=2))
        small = ec.enter_context(tc.tile_pool(name="small", bufs=2))
        ps_tp = ec.enter_context(tc.tile_pool(name="ps_tp", bufs=2, space="PSUM"))
        ps_main = ec.enter_context(tc.tile_pool(name="ps_main", bufs=3, space="PSUM"))
        ps_sc = ec.enter_context(tc.tile_pool(name="ps_sc", bufs=1, space="PSUM"))
        ps_ctx = ec.enter_context(tc.tile_pool(name="ps_ctx", bufs=1, space="PSUM"))

        # ---- identity (for PE transposes) -------------------------------
        id_f = persist.tile([P, P], F32, tag="idf")
        make_identity(nc, id_f[:])
        id_sb = persist.tile([P, P], F32R, tag="id")
        nc.vector.tensor_copy(id_sb[:], id_f[:])
        id_r = id_sb[:]

        # persistent tiles (DMAs emitted below, after the first enc block's)
        w_sb = persist.tile([P, NE, D], F32R, tag="w")
        v_sb = persist.tile([P, ND], F32R, tag="v")
        bias_eb = persist.tile([P, ND], F32, tag="be")
        be_tmp = persist.tile([P, ND], F32, tag="bd")
        decT = persist.tile([P, B_LOC, NE], F32R, tag="decT")
        bias_tot = persist.tile([P, ND, B_LOC], F32, tag="btot")

        # ---- per-block emission helpers ---------------------------------
        def emit_transposes(b, rb):
            """DMA natural enc rows and PE-transpose them to encT layout."""
            nat_t = nat_pool.tile([P, NSUB, E], F32R, tag="nat", name=f"nat_{b}_{rb}")
            src = enc[b, ts(rb, RB), :].rearrange("(a p) e -> p a e", p=P)
            for sub in range(NSUB):
                nc.sync.dma_start(nat_t[:, sub, :], src[:, sub, :])
            tT_t = tT_pool.tile([P, NE, RB], F32R, tag="tT", name=f"tT_{b}_{rb}")
            for ech in range(NE):
                ps_t = ps_tp.tile(
                    [P, NSUB, P], F32R, tag="tp", name=f"ps_t_{b}_{rb}_{ech}"
                )
                for sub in range(NSUB):
                    nc.tensor.transpose(
                        ps_t[:, sub, :], nat_t[:, sub, ts(ech, P)], id_r
                    )
                nc.vector.tensor_copy(
                    tT_t[:, ech, :], ps_t[:].rearrange("p a q -> p (a q)")
                )
            return tT_t

        def emit_mains(b, rb, tT_t, scores_b):
            """Projection matmuls + tanh + v-scores for one r-block."""
            comb_t = comb_pool.tile(
                [P, ND, RB], F32R, tag="comb", name=f"comb_{b}_{rb}"
            )
            for dg in range(ND // 2):
                ps_m = [
                    ps_main.tile(
                        [P, RB], F32, tag="main", name=f"psm_{b}_{rb}_{dg}_{j}"
                    )
                    for j in range(2)
                ]
                for ech in range(NE):
                    for j in range(2):
                        dt = dg * 2 + j
                        nc.tensor.matmul(
                            ps_m[j][:],
                            w_sb[:, ech, ts(dt, P)],
                            tT_t[:, ech, :],
                            start=(ech == 0),
                            stop=(ech == NE - 1),
                        )
                for j in range(2):
                    dt = dg * 2 + j
                    nc.scalar.activation(
                        comb_t[:, dt, :],
                        ps_m[j][:],
                        mybir.ActivationFunctionType.Tanh,
                        bias=bias_tot[:, dt, b : b + 1],
                    )
            ps_s = ps_sc.tile([1, RB], F32, tag="sc", name=f"ps_s_{b}_{rb}")
            for dt in range(ND):
                nc.tensor.matmul(
                    ps_s[:],
                    v_sb[:, dt : dt + 1],
                    comb_t[:, dt, :],
                    start=(dt == 0),
                    stop=(dt == ND - 1),
                )
            nc.vector.tensor_copy(scores_b[0:1, ts(rb, RB)], ps_s[:])

        def emit_softmax(b, scores_b):
            negmax = small.tile([1, 1], F32, tag="negmax", name=f"negmax_{b}")
            nc.vector.reduce_max(
                negmax[:], scores_b[:], axis=mybir.AxisListType.X, negate=True
            )
            ssum = small.tile([1, 1], F32, tag="ssum", name=f"ssum_{b}")
            nc.scalar.activation(
                scores_b[:],
                scores_b[:],
                mybir.ActivationFunctionType.Exp,
                bias=negmax[0:1, 0:1],
                accum_out=ssum[0:1, 0:1],
            )
            rsum = small.tile([1, 1], F32, tag="rsum", name=f"rsum_{b}")
            nc.vector.reciprocal(rsum[:], ssum[:])
            attn_n = scores_b
            nc.vector.tensor_scalar_mul(attn_n[:], scores_b[:], rsum[0:1, 0:1])
            nc.sync.dma_start(attn_o[b : b + 1, :], attn_n[:])
            return attn_n

        def emit_ctx(b, attn_n):
            """attn transpose + context matmuls for batch b (one batch late)."""
            ps_at = ps_tp.tile([P, NSC], F32, tag="tp", name=f"ps_at_{b}")
            for c in range(NSC):
                nc.tensor.transpose(
                    ps_at[:, c : c + 1],
                    attn_n[0:1, ts(c, P)],
                    id_f[0:1, 0:1],
                )
            attnT = small.tile([P, NSC], F32R, tag="attnT", name=f"attnT_{b}")
            nc.vector.tensor_copy(attnT[:], ps_at[:])

            ps_c = ps_ctx.tile([1, E], F32, tag="ctxp", name=f"ps_c_{b}")
            for rb in range(NRB):
                nat2_t = nat2_pool.tile(
                    [P, NSUB, E], F32R, tag="nat2", name=f"nat2_{b}_{rb}"
                )
                src2 = enc[b, ts(rb, RB), :].rearrange("(a p) e -> p a e", p=P)
                for sub in range(NSUB):
                    nc.sync.dma_start(nat2_t[:, sub, :], src2[:, sub, :])
                for sub in range(NSUB):
                    c = rb * NSUB + sub
                    for half in range(2):
                        nc.tensor.matmul(
                            ps_c[0:1, ts(half, 512)],
                            attnT[:, c : c + 1],
                            nat2_t[:, sub, ts(half, 512)],
                            start=(c == 0),
                            stop=(c == NSC - 1),
                        )
            ctx_sb = small.tile([1, E], F32, tag="ctx_sb", name=f"ctx_sb_{b}")
            nc.vector.tensor_copy(ctx_sb[:], ps_c[:])
            nc.sync.dma_start(ctx_o[b : b + 1, :], ctx_sb[:])

        # ---- prologue: first enc block first, then weights --------------
        blocks = [(b, rb) for b in range(B_LOC) for rb in range(NRB)]
        tTs = {blocks[0]: emit_transposes(*blocks[0])}

        for ech in range(NE):
            nc.sync.dma_start(w_sb[:, ech, :], w_enc[ts(ech, P), :])
        nc.sync.dma_start(v_sb[:], v_in.rearrange("(c p) -> p c", p=P))
        nc.sync.dma_start(bias_eb[:], b_enc.rearrange("(c p) -> p c", p=P))
        nc.sync.dma_start(be_tmp[:], b_dec.rearrange("(c p) -> p c", p=P))
        nc.vector.tensor_add(bias_eb[:], bias_eb[:], be_tmp[:])
        for bb in range(B_LOC):
            nc.sync.dma_start(
                decT[:, bb, :], dec[bb, :].rearrange("(c p) -> p c", p=P)
            )

        def emit_dec_projection():  # noqa: E306
            # One psum tile per d-tile: PSUM has_written state is per-BANK,
            # so interleaved accumulation groups in one bank lose terms.
            for dt in range(ND):
                wd_t = wdec_pool.tile(
                    [P, NE, P], F32R, tag="wdec", name=f"wdt_{dt}"
                )
                nc.sync.dma_start(
                    wd_t[:], w_dec[:, ts(dt, P)].rearrange("(c p) d -> p c d", p=P)
                )
                ps_d = ps_tp.tile([P, B_LOC], F32, tag="tp", name=f"psd_{dt}")
                for ech in range(NE):
                    nc.tensor.matmul(
                        ps_d[:],
                        wd_t[:, ech, :],
                        decT[:, :, ech],
                        start=(ech == 0),
                        stop=(ech == NE - 1),
                    )
                # bias_total[d, dt, b] = dec_t[d, b] + (b_enc + b_dec)[d]
                nc.vector.tensor_scalar_add(
                    bias_tot[:, dt, :], ps_d[:], bias_eb[:, dt : dt + 1]
                )

        emit_dec_projection()

        # ---- main loop: transposes pipelined one r-block ahead ----------
        scores = {}
        pending = None
        last = blocks[-1]
        for i, (b, rb) in enumerate(blocks):
            if rb == 0:
                scores[b] = small.tile([1, S], F32, tag="scores", name=f"scores_{b}")
            if i + 1 < len(blocks):
                tTs[blocks[i + 1]] = emit_transposes(*blocks[i + 1])
            emit_mains(b, rb, tTs.pop((b, rb)), scores[b])
            # context of the PREVIOUS batch, early in this batch's phase B:
            # its softmax finished during the last blocks of the previous
            # batch, so PE never stalls on the chain, and the tail shrinks.
            if rb == 0 and pending is not None:
                emit_ctx(*pending)
                pending = None
            if rb == NRB - 1:
                pending = (b, emit_softmax(b, scores.pop(b)))
        emit_ctx(*pending)

    nc.compile()
    return nc


def kernel(
    decoder_hidden,
    encoder_outputs,
    W_enc,
    b_enc,
    W_dec,
    b_dec,
    v,
    b_v=None,
    trace=False,
    **_unused,
):
    decoder_hidden = np.ascontiguousarray(np.asarray(decoder_hidden, dtype=np.float32))
    encoder_outputs = np.ascontiguousarray(np.asarray(encoder_outputs, dtype=np.float32))
    W_enc = np.ascontiguousarray(np.asarray(W_enc, dtype=np.float32))
    b_enc = np.ascontiguousarray(np.asarray(b_enc, dtype=np.float32))
    W_dec = np.ascontiguousarray(np.asarray(W_dec, dtype=np.float32))
    b_dec = np.ascontiguousarray(np.asarray(b_dec, dtype=np.float32))
    v = np.ascontiguousarray(np.asarray(v, dtype=np.float32))

    if "nc" not in _cache:
        _cache["nc"] = _build()
    nc = _cache["nc"]

    in_maps = []
    for c in range(N_CORES):
        sl = slice(c * B_LOC, (c + 1) * B_LOC)
        in_maps.append(
            {
                "dec": decoder_hidden[sl],
                "enc": encoder_outputs[sl],
                "w_enc": W_enc,
                "b_enc": b_enc,
                "w_dec": W_dec,
                "b_dec": b_dec,
                "v": v,
            }
        )

    res = run_bass_kernel_spmd(
        nc, in_maps, core_ids=list(range(N_CORES)), trace=trace
    )
    _cache["exec_time_ns"] = res.exec_time_ns
    _cache["results"] = res.results
    if res.instructions_and_trace:
        _cache["trace_path"] = res.instructions_and_trace[1]
    context = np.concatenate([r["ctx"] for r in res.results], axis=0)
    attn = np.concatenate([r["attn"] for r in res.results], axis=0)
    return (context, attn)


if __name__ == "__main__":
    rng = np.random.default_rng(0)
    inputs = {
        "decoder_hidden": rng.standard_normal((B, D)).astype(np.float32),
        "encoder_outputs": rng.standard_normal((B, S, E)).astype(np.float32),
        "W_enc": rng.uniform(-1 / 32, 1 / 32, (E, D)).astype(np.float32),
        "b_enc": rng.uniform(-1 / 32, 1 / 32, (D,)).astype(np.float32),
        "W_dec": rng.uniform(-1 / 32, 1 / 32, (D, D)).astype(np.float32),
        "b_dec": rng.uniform(-1 / 32, 1 / 32, (D,)).astype(np.float32),
        "v": rng.uniform(-1 / 32, 1 / 32, (D,)).astype(np.float32),
        "b_v": np.float32(0.01),
    }
    ctx, attn = kernel(**inputs)
    print("ctx", ctx.shape, "attn", attn.shape)


# revision 43
# speedup vs baseline: 1.0825x; 1.0458x over previous
"""Bahdanau attention Trainium2 Bass kernel.

Problem (hardcoded): B=32, S=2048, ENC=DEC=1024, fp32.
  enc_t = encoder_outputs @ W_enc + b_enc          # (B,S,D)
  dec_t = decoder_hidden @ W_dec + b_dec           # (B,D)
  combined = tanh(enc_t + dec_t[:,None,:])         # (B,S,D)
  scores = combined . v  (+ b_v)                   # (B,S)  (b_v drops out of softmax)
  attn = softmax(scores, axis=1)                   # (B,S)
  context = attn . encoder_outputs                 # (B,E)
returns (context, attn)

Sharding: data-parallel over batch across 8 NeuronCores (4 batches/core),
projection matrices replicated. No collectives.

Per-core dataflow (all matmuls in float32r = single-pass fp32 at 1 cyc/row):
  - enc natural tiles [s128, e] are PE-transposed to encT [e128, s] tiles,
    pipelined one r-block ahead so the PSUM->SBUF copies hide under matmuls
  - enc_tT[d, r] = sum_e W_enc[e,d]^T encT[e,r] accumulated in PSUM
  - ScalarE applies tanh(psum + (dec_t[d,b]+b_enc+b_dec)) in one pass (per-
    partition bias) -> combined [d128, r]
  - scores[1, r] = v . combined via PE over partitions
  - per-batch softmax along the free dim; attn transposed back via PE
  - context[1, e] = sum_s attnT[s,1] . enc[s, e] (second streamed enc read),
    emitted one batch late so the softmax latency chain hides under the next
    batch's matmuls
"""

import numpy as np
from contextlib import ExitStack

import concourse.bass as bass
import concourse.tile as tile
from concourse import bacc, mybir
from concourse.bass import ts
from concourse.bass_utils import run_bass_kernel_spmd
from concourse.masks import make_identity

N_CORES = 8
B = 32
B_LOC = B // N_CORES   # 4
S = 2048
E = 1024               # ENC
D = 1024               # DEC
P = 128
RB = 512               # r-block (moving free dim of main matmuls)
NRB = S // RB          # 4 r-blocks per batch
NSUB = RB // P         # 4 s-subtiles per r-block
NE = E // P            # 8 e-chunks
ND = D // P            # 8 d-tiles
NSC = S // P           # 16 s-chunks (context phase)

F32 = mybir.dt.float32
F32R = mybir.dt.float32r
F16 = mybir.dt.float16

_cache = {}


def _build():
    nc = bacc.Bacc("TRN2", target_bir_lowering=False, debug=False)

    dec = nc.dram_tensor("dec", [B_LOC, D], F32R, kind="ExternalInput").ap()
    enc = nc.dram_tensor("enc", [B_LOC, S, E], F32R, kind="ExternalInput").ap()
    w_enc = nc.dram_tensor("w_enc", [E, D], F32R, kind="ExternalInput").ap()
    b_enc = nc.dram_tensor("b_enc", [D], F32, kind="ExternalInput").ap()
    w_dec = nc.dram_tensor("w_dec", [D, D], F32R, kind="ExternalInput").ap()
    b_dec = nc.dram_tensor("b_dec", [D], F32, kind="ExternalInput").ap()
    v_in = nc.dram_tensor("v", [D], F32R, kind="ExternalInput").ap()
    ctx_o = nc.dram_tensor("ctx", [B_LOC, E], F32, kind="ExternalOutput").ap()
    attn_o = nc.dram_tensor("attn", [B_LOC, S], F32, kind="ExternalOutput").ap()

    with ExitStack() as ec:
        tc = ec.enter_context(tile.TileContext(nc))
        persist = ec.enter_context(tc.tile_pool(name="persist", bufs=1))
        nat_pool = ec.enter_context(tc.tile_pool(name="nat", bufs=2))
        tT_pool = ec.enter_context(tc.tile_pool(name="tT", bufs=2))
        comb_pool = ec.enter_context(tc.tile_pool(name="comb", bufs=2))
        nat2_pool = ec.enter_context(tc.tile_pool(name="nat2", bufs=2))
        wdec_pool = ec.enter_context(tc.tile_pool(name="wdec", bufs=2))
        small = ec.enter_context(tc.tile_pool(name="small", bufs=2))
        ps_tp = ec.enter_context(tc.tile_pool(name="ps_tp", bufs=2, space="PSUM"))
        ps_main = ec.enter_context(tc.tile_pool(name="ps_main", bufs=3, space="PSUM"))
        ps_sc = ec.enter_context(tc.tile_pool(name="ps_sc", bufs=1, space="PSUM"))
        ps_ctx = ec.enter_context(tc.tile_pool(name="ps_ctx", bufs=1, space="PSUM"))

        # ---- identity (for PE transposes) -------------------------------
        id_f = persist.tile([P, P], F32, tag="idf")
        make_identity(nc, id_f[:])
        id_sb = persist.tile([P, P], F32R, tag="id")
        nc.vector.tensor_copy(id_sb[:], id_f[:])
        id_r = id_sb[:]

        # persistent tiles (DMAs emitted below, after the first enc block's)
        # W_enc and v are cast to f16 (10-bit mantissa ~ fp32r precision;
        # 2-byte dtype enables fast weight load on the PE)
        w_sb = persist.tile([P, NE, D], F16, tag="w")
        v_sb = persist.tile([P, ND], F16, tag="v")
        bias_eb = persist.tile([P, ND], F32, tag="be")
        be_tmp = persist.tile([P, ND], F32, tag="bd")
        decT = persist.tile([P, B_LOC, NE], F32R, tag="decT")
        bias_tot = persist.tile([P, ND, B_LOC], F32, tag="btot")

        # ---- per-block emission helpers ---------------------------------
        def emit_transposes(b, rb):
            """DMA natural enc rows and PE-transpose them to encT layout."""
            nat_t = nat_pool.tile([P, NSUB, E], F32R, tag="nat", name=f"nat_{b}_{rb}")
            src = enc[b, ts(rb, RB), :].rearrange("(a p) e -> p a e", p=P)
            for sub in range(NSUB):
                nc.sync.dma_start(nat_t[:, sub, :], src[:, sub, :])
            tT_t = tT_pool.tile([P, NE, RB], F16, tag="tT", name=f"tT_{b}_{rb}")
            for ech in range(NE):
                ps_t = ps_tp.tile(
                    [P, NSUB, P], F32R, tag="tp", name=f"ps_t_{b}_{rb}_{ech}"
                )
                for sub in range(NSUB):
                    nc.tensor.transpose(
                        ps_t[:, sub, :], nat_t[:, sub, ts(ech, P)], id_r
                    )
                nc.vector.tensor_copy(
                    tT_t[:, ech, :],
                    ps_t[:].rearrange("p a q -> p (a q)").bitcast(F32),
                )
            return tT_t

        def emit_mains(b, rb, tT_t, scores_b):
            """Projection matmuls + tanh + v-scores for one r-block."""
            comb_t = comb_pool.tile(
                [P, ND, RB], F16, tag="comb", name=f"comb_{b}_{rb}"
            )
            for dg in range(ND // 2):
                ps_m = [
                    ps_main.tile(
                        [P, RB], F32, tag="main", name=f"psm_{b}_{rb}_{dg}_{j}"
                    )
                    for j in range(2)
                ]
                for ech in range(NE):
                    for j in range(2):
                        dt = dg * 2 + j
                        nc.tensor.matmul(
                            ps_m[j][:],
                            w_sb[:, ech, ts(dt, P)],
                            tT_t[:, ech, :],
                            start=(ech == 0),
                            stop=(ech == NE - 1),
                        )
                for j in range(2):
                    dt = dg * 2 + j
                    nc.scalar.activation(
                        comb_t[:, dt, :],
                        ps_m[j][:],
                        mybir.ActivationFunctionType.Tanh,
                        bias=bias_tot[:, dt, b : b + 1],
                    )
            ps_s = ps_sc.tile([1, RB], F32, tag="sc", name=f"ps_s_{b}_{rb}")
            for dt in range(ND):
                nc.tensor.matmul(
                    ps_s[:],
                    v_sb[:, dt : dt + 1],
                    comb_t[:, dt, :],
                    start=(dt == 0),
                    stop=(dt == ND - 1),
                )
            nc.vector.tensor_copy(scores_b[0:1, ts(rb, RB)], ps_s[:])

        def emit_softmax(b, scores_b):
            negmax = small.tile([1, 1], F32, tag="negmax", name=f"negmax_{b}")
            nc.vector.reduce_max(
                negmax[:], scores_b[:], axis=mybir.AxisListType.X, negate=True
            )
            ssum = small.tile([1, 1], F32, tag="ssum", name=f"ssum_{b}")
            nc.scalar.activation(
                scores_b[:],
                scores_b[:],
                mybir.ActivationFunctionType.Exp,
                bias=negmax[0:1, 0:1],
                accum_out=ssum[0:1, 0:1],
            )
            rsum = small.tile([1, 1], F32, tag="rsum", name=f"rsum_{b}")
            nc.vector.reciprocal(rsum[:], ssum[:])
            attn_n = scores_b
            nc.vector.tensor_scalar_mul(attn_n[:], scores_b[:], rsum[0:1, 0:1])
            nc.sync.dma_start(attn_o[b : b + 1, :], attn_n[:])
            return attn_n

        def emit_ctx(b, attn_n):
            """attn transpose + context matmuls for batch b (one batch late)."""
            ps_at = ps_tp.tile([P, NSC], F32, tag="tp", name=f"ps_at_{b}")
            for c in range(NSC):
                nc.tensor.transpose(
                    ps_at[:, c : c + 1],
                    attn_n[0:1, ts(c, P)],
                    id_f[0:1, 0:1],
                )
            attnT = small.tile([P, NSC], F32R, tag="attnT", name=f"attnT_{b}")
            nc.vector.tensor_copy(attnT[:], ps_at[:])

            ps_c = ps_ctx.tile([1, E], F32, tag="ctxp", name=f"ps_c_{b}")
            for rb in range(NRB):
                nat2_t = nat2_pool.tile(
                    [P, NSUB, E], F32R, tag="nat2", name=f"nat2_{b}_{rb}"
                )
                src2 = enc[b, ts(rb, RB), :].rearrange("(a p) e -> p a e", p=P)
                for sub in range(NSUB):
                    nc.sync.dma_start(nat2_t[:, sub, :], src2[:, sub, :])
                for sub in range(NSUB):
                    c = rb * NSUB + sub
                    for half in range(2):
                        nc.tensor.matmul(
                            ps_c[0:1, ts(half, 512)],
                            attnT[:, c : c + 1],
                            nat2_t[:, sub, ts(half, 512)],
                            start=(c == 0),
                            stop=(c == NSC - 1),
                        )
            ctx_sb = small.tile([1, E], F32, tag="ctx_sb", name=f"ctx_sb_{b}")
            nc.vector.tensor_copy(ctx_sb[:], ps_c[:])
            nc.sync.dma_start(ctx_o[b : b + 1, :], ctx_sb[:])

        # ---- prologue: first enc block first, then weights --------------
        blocks = [(b, rb) for b in range(B_LOC) for rb in range(NRB)]
        tTs = {blocks[0]: emit_transposes(*blocks[0])}

        for ech in range(NE):
            wst = wdec_pool.tile([P, D], F32R, tag="wst", name=f"wst_{ech}")
            nc.sync.dma_start(wst[:], w_enc[ts(ech, P), :])
            nc.vector.tensor_copy(w_sb[:, ech, :], wst[:].bitcast(F32))
        vst = persist.tile([P, ND], F32, tag="vst")
        nc.sync.dma_start(vst[:], v_in.bitcast(F32).rearrange("(c p) -> p c", p=P))
        nc.vector.tensor_copy(v_sb[:], vst[:])
        nc.sync.dma_start(bias_eb[:], b_enc.rearrange("(c p) -> p c", p=P))
        nc.sync.dma_start(be_tmp[:], b_dec.rearrange("(c p) -> p c", p=P))
        nc.vector.tensor_add(bias_eb[:], bias_eb[:], be_tmp[:])
        for bb in range(B_LOC):
            nc.sync.dma_start(
                decT[:, bb, :], dec[bb, :].rearrange("(c p) -> p c", p=P)
            )

        def emit_dec_projection():  # noqa: E306
            # One psum tile per d-tile: PSUM has_written state is per-BANK,
            # so interleaved accumulation groups in one bank lose terms.
            for dt in range(ND):
                wd_t = wdec_pool.tile(
                    [P, NE, P], F32R, tag="wdec", name=f"wdt_{dt}"
                )
                nc.sync.dma_start(
                    wd_t[:], w_dec[:, ts(dt, P)].rearrange("(c p) d -> p c d", p=P)
                )
                ps_d = ps_tp.tile([P, B_LOC], F32, tag="tp", name=f"psd_{dt}")
                for ech in range(NE):
                    nc.tensor.matmul(
                        ps_d[:],
                        wd_t[:, ech, :],
                        decT[:, :, ech],
                        start=(ech == 0),
                        stop=(ech == NE - 1),
                    )
                # bias_total[d, dt, b] = dec_t[d, b] + (b_enc + b_dec)[d]
                nc.vector.tensor_scalar_add(
                    bias_tot[:, dt, :], ps_d[:], bias_eb[:, dt : dt + 1]
                )

        emit_dec_projection()

        # ---- main loop: transposes pipelined one r-block ahead ----------
        scores = {}
        pending = None
        last = blocks[-1]
        for i, (b, rb) in enumerate(blocks):
            if rb == 0:
                scores[b] = small.tile([1, S], F32, tag="scores", name=f"scores_{b}")
            if i + 1 < len(blocks):
                tTs[blocks[i + 1]] = emit_transposes(*blocks[i + 1])
            emit_mains(b, rb, tTs.pop((b, rb)), scores[b])
            # context of the PREVIOUS batch, early in this batch's phase B:
            # its softmax finished during the last blocks of the previous
            # batch, so PE never stalls on the chain, and the tail shrinks.
            if rb == 0 and pending is not None:
                emit_ctx(*pending)
                pending = None
            if rb == NRB - 1:
                pending = (b, emit_softmax(b, scores.pop(b)))
        emit_ctx(*pending)

    nc.compile()
    return nc


def kernel(
    decoder_hidden,
    encoder_outputs,
    W_enc,
    b_enc,
    W_dec,
    b_dec,
    v,
    b_v=None,
    trace=False,
    **_unused,
):
    decoder_hidden = np.ascontiguousarray(np.asarray(decoder_hidden, dtype=np.float32))
    encoder_outputs = np.ascontiguousarray(np.asarray(encoder_outputs, dtype=np.float32))
    W_enc = np.ascontiguousarray(np.asarray(W_enc, dtype=np.float32))
    b_enc = np.ascontiguousarray(np.asarray(b_enc, dtype=np.float32))
    W_dec = np.ascontiguousarray(np.asarray(W_dec, dtype=np.float32))
    b_dec = np.ascontiguousarray(np.asarray(b_dec, dtype=np.float32))
    v = np.ascontiguousarray(np.asarray(v, dtype=np.float32))

    if "nc" not in _cache:
        _cache["nc"] = _build()
    nc = _cache["nc"]

    in_maps = []
    for c in range(N_CORES):
        sl = slice(c * B_LOC, (c + 1) * B_LOC)
        in_maps.append(
            {
                "dec": decoder_hidden[sl],
                "enc": encoder_outputs[sl],
                "w_enc": W_enc,
                "b_enc": b_enc,
                "w_dec": W_dec,
                "b_dec": b_dec,
                "v": v,
            }
        )

    res = run_bass_kernel_spmd(
        nc, in_maps, core_ids=list(range(N_CORES)), trace=trace
    )
    _cache["exec_time_ns"] = res.exec_time_ns
    _cache["results"] = res.results
    if res.instructions_and_trace:
        _cache["trace_path"] = res.instructions_and_trace[1]
    context = np.concatenate([r["ctx"] for r in res.results], axis=0)
    attn = np.concatenate([r["attn"] for r in res.results], axis=0)
    return (context, attn)


if __name__ == "__main__":
    rng = np.random.default_rng(0)
    inputs = {
        "decoder_hidden": rng.standard_normal((B, D)).astype(np.float32),
        "encoder_outputs": rng.standard_normal((B, S, E)).astype(np.float32),
        "W_enc": rng.uniform(-1 / 32, 1 / 32, (E, D)).astype(np.float32),
        "b_enc": rng.uniform(-1 / 32, 1 / 32, (D,)).astype(np.float32),
        "W_dec": rng.uniform(-1 / 32, 1 / 32, (D, D)).astype(np.float32),
        "b_dec": rng.uniform(-1 / 32, 1 / 32, (D,)).astype(np.float32),
        "v": rng.uniform(-1 / 32, 1 / 32, (D,)).astype(np.float32),
        "b_v": np.float32(0.01),
    }
    ctx, attn = kernel(**inputs)
    print("ctx", ctx.shape, "attn", attn.shape)


# revision 46
# speedup vs baseline: 1.1213x; 1.0358x over previous
"""Bahdanau attention Trainium2 Bass kernel.

Problem (hardcoded): B=32, S=2048, ENC=DEC=1024, fp32.
  enc_t = encoder_outputs @ W_enc + b_enc          # (B,S,D)
  dec_t = decoder_hidden @ W_dec + b_dec           # (B,D)
  combined = tanh(enc_t + dec_t[:,None,:])         # (B,S,D)
  scores = combined . v  (+ b_v)                   # (B,S)  (b_v drops out of softmax)
  attn = softmax(scores, axis=1)                   # (B,S)
  context = attn . encoder_outputs                 # (B,E)
returns (context, attn)

Sharding: data-parallel over batch across 8 NeuronCores (4 batches/core),
projection matrices replicated. No collectives.

Per-core dataflow (all matmuls in float32r = single-pass fp32 at 1 cyc/row):
  - enc natural tiles [s128, e] are PE-transposed to encT [e128, s] tiles,
    pipelined one r-block ahead so the PSUM->SBUF copies hide under matmuls
  - enc_tT[d, r] = sum_e W_enc[e,d]^T encT[e,r] accumulated in PSUM
  - ScalarE applies tanh(psum + (dec_t[d,b]+b_enc+b_dec)) in one pass (per-
    partition bias) -> combined [d128, r]
  - scores[1, r] = v . combined via PE over partitions
  - per-batch softmax along the free dim; attn transposed back via PE
  - context[1, e] = sum_s attnT[s,1] . enc[s, e] (second streamed enc read),
    emitted one batch late so the softmax latency chain hides under the next
    batch's matmuls
"""

import numpy as np
from contextlib import ExitStack

import concourse.bass as bass
import concourse.tile as tile
from concourse import bacc, mybir
from concourse.bass import ts
from concourse.bass_utils import run_bass_kernel_spmd
from concourse.masks import make_identity

N_CORES = 8
B = 32
B_LOC = B // N_CORES   # 4
S = 2048
E = 1024               # ENC
D = 1024               # DEC
P = 128
RB = 512               # r-block (moving free dim of main matmuls)
NRB = S // RB          # 4 r-blocks per batch
NSUB = RB // P         # 4 s-subtiles per r-block
NE = E // P            # 8 e-chunks
ND = D // P            # 8 d-tiles
NSC = S // P           # 16 s-chunks (context phase)

F32 = mybir.dt.float32
F32R = mybir.dt.float32r
F16 = mybir.dt.float16

_cache = {}


def _build():
    nc = bacc.Bacc("TRN2", target_bir_lowering=False, debug=False)

    dec = nc.dram_tensor("dec", [B_LOC, D], F32R, kind="ExternalInput").ap()
    enc = nc.dram_tensor("enc", [B_LOC, S, E], F32R, kind="ExternalInput").ap()
    w_enc = nc.dram_tensor("w_enc", [E, D], F32R, kind="ExternalInput").ap()
    b_enc = nc.dram_tensor("b_enc", [D], F32, kind="ExternalInput").ap()
    w_dec = nc.dram_tensor("w_dec", [D, D], F32R, kind="ExternalInput").ap()
    b_dec = nc.dram_tensor("b_dec", [D], F32, kind="ExternalInput").ap()
    v_in = nc.dram_tensor("v", [D], F32R, kind="ExternalInput").ap()
    ctx_o = nc.dram_tensor("ctx", [B_LOC, E], F32, kind="ExternalOutput").ap()
    attn_o = nc.dram_tensor("attn", [B_LOC, S], F32, kind="ExternalOutput").ap()

    with ExitStack() as ec:
        tc = ec.enter_context(tile.TileContext(nc))
        persist = ec.enter_context(tc.tile_pool(name="persist", bufs=1))
        nat_pool = ec.enter_context(tc.tile_pool(name="nat", bufs=2))
        n16_pool = ec.enter_context(tc.tile_pool(name="n16", bufs=2))
        tT_pool = ec.enter_context(tc.tile_pool(name="tT", bufs=2))
        comb_pool = ec.enter_context(tc.tile_pool(name="comb", bufs=2))
        nat2_pool = ec.enter_context(tc.tile_pool(name="nat2", bufs=2))
        wdec_pool = ec.enter_context(tc.tile_pool(name="wdec", bufs=2))
        small = ec.enter_context(tc.tile_pool(name="small", bufs=2))
        ps_tp = ec.enter_context(tc.tile_pool(name="ps_tp", bufs=2, space="PSUM"))
        ps_main = ec.enter_context(tc.tile_pool(name="ps_main", bufs=3, space="PSUM"))
        ps_sc = ec.enter_context(tc.tile_pool(name="ps_sc", bufs=1, space="PSUM"))
        ps_ctx = ec.enter_context(tc.tile_pool(name="ps_ctx", bufs=1, space="PSUM"))

        # ---- identity (for PE transposes) -------------------------------
        id_f = persist.tile([P, P], F32, tag="idf")
        make_identity(nc, id_f[:])
        id16 = persist.tile([P, P], F16, tag="id16")
        nc.vector.tensor_copy(id16[:], id_f[:])

        # persistent tiles (DMAs emitted below, after the first enc block's)
        # W_enc and v are cast to f16 (10-bit mantissa ~ fp32r precision;
        # 2-byte dtype enables fast weight load on the PE)
        w_sb = persist.tile([P, NE, D], F16, tag="w")
        v_sb = persist.tile([P, ND], F16, tag="v")
        bias_eb = persist.tile([P, ND], F32, tag="be")
        be_tmp = persist.tile([P, ND], F32, tag="bd")
        decT = persist.tile([P, B_LOC, NE], F32R, tag="decT")
        bias_tot = persist.tile([P, ND, B_LOC], F32, tag="btot")

        # ---- per-block emission helpers ---------------------------------
        def emit_transposes(b, rb):
            """DMA natural enc rows, cast to f16, PE-transpose to encT."""
            nat_t = nat_pool.tile([P, NSUB, E], F32, tag="nat", name=f"nat_{b}_{rb}")
            src = enc[b, ts(rb, RB), :].bitcast(F32).rearrange("(a p) e -> p a e", p=P)
            for sub in range(NSUB):
                nc.sync.dma_start(nat_t[:, sub, :], src[:, sub, :])
            nat16 = n16_pool.tile([P, NSUB, E], F16, tag="n16", name=f"n16_{b}_{rb}")
            for sub in range(NSUB):
                nc.vector.tensor_copy(nat16[:, sub, :], nat_t[:, sub, :])
            tT_t = tT_pool.tile([P, NE, RB], F16, tag="tT", name=f"tT_{b}_{rb}")
            for ech in range(NE):
                ps_t = ps_tp.tile(
                    [P, NSUB, P], F16, tag="tp", name=f"ps_t_{b}_{rb}_{ech}"
                )
                for sub in range(NSUB):
                    nc.tensor.transpose(
                        ps_t[:, sub, :], nat16[:, sub, ts(ech, P)], id16[:]
                    )
                nc.vector.tensor_copy(
                    tT_t[:, ech, :],
                    ps_t[:].rearrange("p a q -> p (a q)"),
                )
            return tT_t

        def emit_mains(b, rb, tT_t, scores_b):
            """Projection matmuls + tanh + v-scores for one r-block."""
            comb_t = comb_pool.tile(
                [P, ND, RB], F16, tag="comb", name=f"comb_{b}_{rb}"
            )
            for dg in range(ND // 2):
                ps_m = [
                    ps_main.tile(
                        [P, RB], F32, tag="main", name=f"psm_{b}_{rb}_{dg}_{j}"
                    )
                    for j in range(2)
                ]
                for ech in range(NE):
                    for j in range(2):
                        dt = dg * 2 + j
                        nc.tensor.matmul(
                            ps_m[j][:],
                            w_sb[:, ech, ts(dt, P)],
                            tT_t[:, ech, :],
                            start=(ech == 0),
                            stop=(ech == NE - 1),
                        )
                for j in range(2):
                    dt = dg * 2 + j
                    nc.scalar.activation(
                        comb_t[:, dt, :],
                        ps_m[j][:],
                        mybir.ActivationFunctionType.Tanh,
                        bias=bias_tot[:, dt, b : b + 1],
                    )
            ps_s = ps_sc.tile([1, RB], F32, tag="sc", name=f"ps_s_{b}_{rb}")
            for dt in range(ND):
                nc.tensor.matmul(
                    ps_s[:],
                    v_sb[:, dt : dt + 1],
                    comb_t[:, dt, :],
                    start=(dt == 0),
                    stop=(dt == ND - 1),
                )
            nc.vector.tensor_copy(scores_b[0:1, ts(rb, RB)], ps_s[:])

        def emit_softmax(b, scores_b):
            negmax = small.tile([1, 1], F32, tag="negmax", name=f"negmax_{b}")
            nc.vector.reduce_max(
                negmax[:], scores_b[:], axis=mybir.AxisListType.X, negate=True
            )
            ssum = small.tile([1, 1], F32, tag="ssum", name=f"ssum_{b}")
            nc.scalar.activation(
                scores_b[:],
                scores_b[:],
                mybir.ActivationFunctionType.Exp,
                bias=negmax[0:1, 0:1],
                accum_out=ssum[0:1, 0:1],
            )
            rsum = small.tile([1, 1], F32, tag="rsum", name=f"rsum_{b}")
            nc.vector.reciprocal(rsum[:], ssum[:])
            attn_n = scores_b
            nc.vector.tensor_scalar_mul(attn_n[:], scores_b[:], rsum[0:1, 0:1])
            nc.sync.dma_start(attn_o[b : b + 1, :], attn_n[:])
            return attn_n

        def emit_ctx(b, attn_n):
            """attn transpose + context matmuls for batch b (one batch late)."""
            ps_at = ps_tp.tile([P, NSC], F32, tag="tp", name=f"ps_at_{b}")
            for c in range(NSC):
                nc.tensor.transpose(
                    ps_at[:, c : c + 1],
                    attn_n[0:1, ts(c, P)],
                    id_f[0:1, 0:1],
                )
            attnT = small.tile([P, NSC], F32R, tag="attnT", name=f"attnT_{b}")
            nc.vector.tensor_copy(attnT[:], ps_at[:])

            ps_c = ps_ctx.tile([1, E], F32, tag="ctxp", name=f"ps_c_{b}")
            for rb in range(NRB):
                nat2_t = nat2_pool.tile(
                    [P, NSUB, E], F32R, tag="nat2", name=f"nat2_{b}_{rb}"
                )
                src2 = enc[b, ts(rb, RB), :].rearrange("(a p) e -> p a e", p=P)
                for sub in range(NSUB):
                    nc.sync.dma_start(nat2_t[:, sub, :], src2[:, sub, :])
                for sub in range(NSUB):
                    c = rb * NSUB + sub
                    for half in range(2):
                        nc.tensor.matmul(
                            ps_c[0:1, ts(half, 512)],
                            attnT[:, c : c + 1],
                            nat2_t[:, sub, ts(half, 512)],
                            start=(c == 0),
                            stop=(c == NSC - 1),
                        )
            ctx_sb = small.tile([1, E], F32, tag="ctx_sb", name=f"ctx_sb_{b}")
            nc.vector.tensor_copy(ctx_sb[:], ps_c[:])
            nc.sync.dma_start(ctx_o[b : b + 1, :], ctx_sb[:])

        # ---- prologue: first enc block first, then weights --------------
        blocks = [(b, rb) for b in range(B_LOC) for rb in range(NRB)]
        tTs = {blocks[0]: emit_transposes(*blocks[0])}

        for ech in range(NE):
            wst = wdec_pool.tile([P, D], F32R, tag="wst", name=f"wst_{ech}")
            nc.sync.dma_start(wst[:], w_enc[ts(ech, P), :])
            nc.vector.tensor_copy(w_sb[:, ech, :], wst[:].bitcast(F32))
        vst = persist.tile([P, ND], F32, tag="vst")
        nc.sync.dma_start(vst[:], v_in.bitcast(F32).rearrange("(c p) -> p c", p=P))
        nc.vector.tensor_copy(v_sb[:], vst[:])
        nc.sync.dma_start(bias_eb[:], b_enc.rearrange("(c p) -> p c", p=P))
        nc.sync.dma_start(be_tmp[:], b_dec.rearrange("(c p) -> p c", p=P))
        nc.vector.tensor_add(bias_eb[:], bias_eb[:], be_tmp[:])
        for bb in range(B_LOC):
            nc.sync.dma_start(
                decT[:, bb, :], dec[bb, :].rearrange("(c p) -> p c", p=P)
            )

        def emit_dec_projection():  # noqa: E306
            # One psum tile per d-tile: PSUM has_written state is per-BANK,
            # so interleaved accumulation groups in one bank lose terms.
            for dt in range(ND):
                wd_t = wdec_pool.tile(
                    [P, NE, P], F32R, tag="wdec", name=f"wdt_{dt}"
                )
                nc.sync.dma_start(
                    wd_t[:], w_dec[:, ts(dt, P)].rearrange("(c p) d -> p c d", p=P)
                )
                ps_d = ps_tp.tile([P, B_LOC], F32, tag="tp", name=f"psd_{dt}")
                for ech in range(NE):
                    nc.tensor.matmul(
                        ps_d[:],
                        wd_t[:, ech, :],
                        decT[:, :, ech],
                        start=(ech == 0),
                        stop=(ech == NE - 1),
                    )
                # bias_total[d, dt, b] = dec_t[d, b] + (b_enc + b_dec)[d]
                nc.vector.tensor_scalar_add(
                    bias_tot[:, dt, :], ps_d[:], bias_eb[:, dt : dt + 1]
                )

        emit_dec_projection()

        # ---- main loop: transposes pipelined one r-block ahead ----------
        scores = {}
        pending = None
        last = blocks[-1]
        for i, (b, rb) in enumerate(blocks):
            if rb == 0:
                scores[b] = small.tile([1, S], F32, tag="scores", name=f"scores_{b}")
            if i + 1 < len(blocks):
                tTs[blocks[i + 1]] = emit_transposes(*blocks[i + 1])
            emit_mains(b, rb, tTs.pop((b, rb)), scores[b])
            # context of the PREVIOUS batch, early in this batch's phase B:
            # its softmax finished during the last blocks of the previous
            # batch, so PE never stalls on the chain, and the tail shrinks.
            if rb == 0 and pending is not None:
                emit_ctx(*pending)
                pending = None
            if rb == NRB - 1:
                pending = (b, emit_softmax(b, scores.pop(b)))
        emit_ctx(*pending)

    nc.compile()
    return nc


def kernel(
    decoder_hidden,
    encoder_outputs,
    W_enc,
    b_enc,
    W_dec,
    b_dec,
    v,
    b_v=None,
    trace=False,
    **_unused,
):
    decoder_hidden = np.ascontiguousarray(np.asarray(decoder_hidden, dtype=np.float32))
    encoder_outputs = np.ascontiguousarray(np.asarray(encoder_outputs, dtype=np.float32))
    W_enc = np.ascontiguousarray(np.asarray(W_enc, dtype=np.float32))
    b_enc = np.ascontiguousarray(np.asarray(b_enc, dtype=np.float32))
    W_dec = np.ascontiguousarray(np.asarray(W_dec, dtype=np.float32))
    b_dec = np.ascontiguousarray(np.asarray(b_dec, dtype=np.float32))
    v = np.ascontiguousarray(np.asarray(v, dtype=np.float32))

    if "nc" not in _cache:
        _cache["nc"] = _build()
    nc = _cache["nc"]

    in_maps = []
    for c in range(N_CORES):
        sl = slice(c * B_LOC, (c + 1) * B_LOC)
        in_maps.append(
            {
                "dec": decoder_hidden[sl],
                "enc": encoder_outputs[sl],
                "w_enc": W_enc,
                "b_enc": b_enc,
                "w_dec": W_dec,
                "b_dec": b_dec,
                "v": v,
            }
        )

    res = run_bass_kernel_spmd(
        nc, in_maps, core_ids=list(range(N_CORES)), trace=trace
    )
    _cache["exec_time_ns"] = res.exec_time_ns
    _cache["results"] = res.results
    if res.instructions_and_trace:
        _cache["trace_path"] = res.instructions_and_trace[1]
    context = np.concatenate([r["ctx"] for r in res.results], axis=0)
    attn = np.concatenate([r["attn"] for r in res.results], axis=0)
    return (context, attn)


if __name__ == "__main__":
    rng = np.random.default_rng(0)
    inputs = {
        "decoder_hidden": rng.standard_normal((B, D)).astype(np.float32),
        "encoder_outputs": rng.standard_normal((B, S, E)).astype(np.float32),
        "W_enc": rng.uniform(-1 / 32, 1 / 32, (E, D)).astype(np.float32),
        "b_enc": rng.uniform(-1 / 32, 1 / 32, (D,)).astype(np.float32),
        "W_dec": rng.uniform(-1 / 32, 1 / 32, (D, D)).astype(np.float32),
        "b_dec": rng.uniform(-1 / 32, 1 / 32, (D,)).astype(np.float32),
        "v": rng.uniform(-1 / 32, 1 / 32, (D,)).astype(np.float32),
        "b_v": np.float32(0.01),
    }
    ctx, attn = kernel(**inputs)
    print("ctx", ctx.shape, "attn", attn.shape)
